# revision 9
# baseline (speedup 1.0000x reference)
"""BiLSTM language model kernel for Trainium2 (8 NeuronCores), v2.

Sharding: data-parallel over batch (B=32 -> 4 per core). Each core runs the
full bidirectional LSTM scan for its batch slice and the full-vocab output
projection + log-softmax for its tokens locally (no collectives).

v2 design (vs the two-matmul-pass baseline):
  - tanh-only scan: sigmoid(x) = (tanh(x/2)+1)/2 with all scale factors
    folded into host-prepped weights (W halved for f/i/o, h-columns halved
    again since the stored hidden is 2h, cell state tracked as s=2C). The
    whole kernel then uses one ACT table set (exp_and_others: tanh+exp), so
    scan activations and projection exps interleave with no table reloads.
  - bf16 scan weights/state: the [48,112] stationary loads once (LDW dedup),
    per-step matmul is a cheap bf16 2-block stream.
  - single projection pass per token tile: matmul -> ACT Exp (PSUM->SBUF
    bf16 cache + f32 accum = per-group sum). After all groups, pass B is a
    DVE tensor_scalar on the uint16-bitcast cache at 4x: out = bits*(ln2/128)
    - c_p, the bit-hack log. The cache-side and sum-side mantissa biases
    cancel, so c_p = float(bits32(S)) * ln2/2^23 in one DVE op (no ACT Ln).
  - bf16 DRAM output (halved DMA); host casts to f32.
  - token tiles middle-out: tile 0 = s 16..47 is ready at scan step 47, so
    its first exp groups are interleaved into scan steps 48..63; tile 1 =
    s {0..15, 48..63} follows. Phases pipeline as A(0) -> B(0) || A(1) -> B(1).
  - projection weights live in two 33-row bands (partitions 0:33 and 64:97
    of one [97, 26112] bf16 tile) to respect the per-partition SBUF budget;
    the stationary concat features are duplicated at partitions 64:97 so
    base partitions match per band.

DVE 2-input ops need both SBUF inputs at the same base partition; gates land
as tanh_f@0, tanh_i@32, tanh_o@64 with ct parked @32 and tanh(C) @64.
"""

import numpy as np
from contextlib import ExitStack

from concourse import inst_simplify

import concourse.bass as bass
import concourse.mybir as mybir
import concourse.tile as tile
from concourse import bacc
from concourse.masks import make_identity

F32 = mybir.dt.float32
BF16 = mybir.dt.bfloat16
U16 = mybir.dt.uint16
I32 = mybir.dt.int32
AF = mybir.ActivationFunctionType
ALU = mybir.AluOpType

S = 64          # sequence length
B = 32          # full batch
V = 50257       # vocab
HID = 16
EMB = 32
NCORES = 8
BL = B // NCORES          # batch per core = 4
T = S * BL                # tokens per core = 256
KC = EMB + HID            # 48
GP = 112                  # gate rows: tanh_f@0, tanh_i@32, tanh_o@64, tanh_c@96
REV = (S + 1) * BL        # column offset of reverse region in comb = 260
GW = 1536                 # vocab columns per group (3 PSUM banks)
BAND = 26112              # vocab columns in band 0 (= 17 groups)

LN2 = float(np.log(2.0))

# token-tile permutation: dev row r <-> (s = PERM_S[r//4], b = r%4).
# tile 0 (rows 0:128) = s 16..47 -- ready at scan step 47 (middle-out).
PERM_S = list(range(16, 48)) + list(range(0, 16)) + list(range(48, 64))

# groups: (band, local_start, width, global_start, out_width)
GROUPS = []
for g in range(17):
    GROUPS.append((0, g * GW, GW, g * GW, GW))
for g in range(15):
    GROUPS.append((1, g * GW, GW, BAND + g * GW, GW))
# last group padded to even width 1106; the phantom zero-weight column adds
# exp(0)=1 to S (~2e-5 relative) and is not DMA'd out.
GROUPS.append((1, 15 * GW, 1106, BAND + 15 * GW, 1105))
NG = len(GROUPS)          # 33


def _two_block(ap2d, col_a, col_b, width):
    """AP selecting two `width`-column blocks [P, 2, width] of a 2D sbuf AP."""
    base = ap2d
    return bass.AP(
        base.tensor,
        base.offset + col_a,
        [base.ap[0], [col_b - col_a, 2], [1, width]],
    )


def build_nc():
    nc = bacc.Bacc("TRN2", target_bir_lowering=False, debug=False)

    # ---------------- DRAM I/O ----------------
    d_emb = nc.dram_tensor("emb_table", [V, EMB], F32, kind="ExternalInput")
    d_idx = nc.dram_tensor("idx", [T, 1], I32, kind="ExternalInput")
    d_wcomb = nc.dram_tensor("w_combT", [KC, GP], BF16, kind="ExternalInput")
    d_bcell = nc.dram_tensor("b_cell", [GP, 1], F32, kind="ExternalInput")
    d_h0 = nc.dram_tensor("h0", [HID, BL], BF16, kind="ExternalInput")
    d_s0 = nc.dram_tensor("s0", [HID, 2 * BL], F32, kind="ExternalInput")
    d_wlo = nc.dram_tensor("w_lo", [33, BAND], BF16, kind="ExternalInput")
    d_whi = nc.dram_tensor("w_hi", [33, BAND], BF16, kind="ExternalInput")
    d_out = nc.dram_tensor("out", [T, V], BF16, kind="ExternalOutput")

    with tile.TileContext(nc) as tc, ExitStack() as ctx:
        singles = ctx.enter_context(tc.tile_pool(name="singles", bufs=1))
        # projection PSUM first so it gets banks disjoint from the scan pool
        psP = ctx.enter_context(tc.tile_pool(name="psP", bufs=2, space="PSUM"))
        cachep = ctx.enter_context(tc.tile_pool(name="cache", bufs=NG + 1))
        obp = ctx.enter_context(tc.tile_pool(name="ob", bufs=4))
        scp = ctx.enter_context(tc.tile_pool(name="scr", bufs=2))

        # gather inputs first: the idx DMAs lead the sync queue and the
        # indirect gathers lead the gpsimd queue, so the embedding rows are
        # in flight while everything else initializes.
        gat = ctx.enter_context(tc.tile_pool(name="pgather", bufs=2))
        idx_sbs, embgs = [], []
        for half in range(2):
            idx_sb = gat.tile([128, 1], I32, tag="idx", name=f"idx{half}")
            nc.sync.dma_start(idx_sb, d_idx.ap()[half * 128 : (half + 1) * 128, :])
            idx_sbs.append(idx_sb)
        for half in range(2):
            embg = gat.tile([128, EMB], F32, tag="embg", name=f"embg{half}")
            nc.gpsimd.indirect_dma_start(
                out=embg,
                out_offset=None,
                in_=d_emb.ap(),
                in_offset=bass.IndirectOffsetOnAxis(ap=idx_sbs[half][:, :1], axis=0),
            )
            embgs.append(embg)

        ident = singles.tile([128, 128], F32)
        make_identity(nc, ident)

        w_combT = singles.tile([KC, GP], BF16)
        nc.sync.dma_start(w_combT, d_wcomb.ap())
        b_cell = singles.tile([GP, 1], F32)
        nc.sync.dma_start(b_cell, d_bcell.ap())

        # comb: [48, 520] bf16; cols 0..260 fwd blocks 0..64, cols 260..520
        # rev blocks 0..64. rows 0-31 = x (embT), rows 32-47 = 2h state.
        comb = singles.tile([KC, 2 * REV], BF16)
        # concat features, bf16: rows 0-15 lefts, 16-31 rights, 32 ones; the
        # same three bands duplicated at rows 64-96 for band-1 matmuls.
        ca2 = singles.tile([97, T], BF16)
        nc.vector.memset(ca2[32:33, :], 1.0)
        nc.vector.memset(ca2[96:97, :], 1.0)

        # s = 2C state [16, 8] f32 (fwd | rev)
        s_st = singles.tile([HID, 2 * BL], F32)
        nc.sync.dma_start(s_st, d_s0.ap())
        nc.sync.dma_start(comb[EMB:KC, 0:BL], d_h0.ap())            # fwd block 0
        nc.sync.dma_start(comb[EMB:KC, 2 * REV - BL : 2 * REV], d_h0.ap())  # rev 64

        embT = singles.tile([EMB, T], BF16)
        # two-band projection weights: rows 0:33 = vocab [0, 26112),
        # rows 64:97 = vocab [26112, 50257) zero-padded.
        w_sb = singles.tile([97, BAND], BF16)
        nc.scalar.dma_start(w_sb[0:33, :], d_wlo.ap())
        nc.scalar.dma_start(w_sb[64:97, :], d_whi.ap())

        # per-group exp sums; col = tile*NG + g
        partials = singles.tile([128, 2 * NG], F32)
        # c_p = float(bits32(S)) * ln2/2^23 per token, one col per tile
        cp = singles.tile([128, 2], F32)

        # ---------------- embedding transpose via PE ----------------
        with tc.tile_pool(name="ps_misc", bufs=2, space="PSUM") as psm:
            for half in range(2):
                ps_tr = psm.tile([EMB, 128], F32)
                nc.tensor.transpose(ps_tr, embgs[half], ident)
                nc.vector.tensor_copy(embT[:, half * 128 : (half + 1) * 128], ps_tr)

        # x parts of comb: fwd block t = token t; rev block m+1 = token m
        nc.vector.tensor_copy(comb[0:EMB, 0:T], embT)
        nc.vector.tensor_copy(comb[0:EMB, REV + BL : REV + BL + T], embT)

        # ---------------- scan machinery ----------------
        ssb = ctx.enter_context(tc.tile_pool(name="scan_sb", bufs=6))
        sps = ctx.enter_context(tc.tile_pool(name="scan_ps", bufs=2, space="PSUM"))

        def scan_step(t):
            f_col = BL * t                       # fwd block t
            r_col = REV + BL * (S - t)           # rev block 64-t
            rhs = _two_block(comb[:, :], f_col, r_col, BL)
            g_ps = sps.tile([GP, 2 * BL], F32)
            nc.tensor.matmul(g_ps, w_combT, rhs, start=True, stop=True)

            # tg rows: tanh(zf/2)@0, tanh(zi/2)@32, tanh(zo/2)@64
            tg = ssb.tile([96, 2 * BL], F32, tag="tg")
            nc.scalar.activation(
                tg, g_ps[0:96, :], AF.Tanh, bias=b_cell[0:96, :]
            )
            ct = ssb.tile([48, 2 * BL], F32, tag="ct")   # tanh(z_C) @ 32
            nc.scalar.activation(
                ct[32:48, :], g_ps[96:GP, :], AF.Tanh, bias=b_cell[96:GP, :]
            )

            # s_new = 2C_new = 0.5*(tf+1)*s + (ti+1)*ct
            p = ssb.tile([48, 2 * BL], F32, tag="p")
            nc.vector.scalar_tensor_tensor(
                p[32:48, :], tg[0:HID, :], 1.0, s_st[:, :], ALU.add, ALU.mult
            )
            q = ssb.tile([48, 2 * BL], F32, tag="q")
            nc.vector.scalar_tensor_tensor(
                q[32:48, :], tg[32 : 32 + HID, :], 1.0, ct[32:48, :],
                ALU.add, ALU.mult,
            )
            nc.vector.scalar_tensor_tensor(
                s_st[:, :], p[32:48, :], 0.5, q[32:48, :], ALU.mult, ALU.add
            )

            th = ssb.tile([80, 2 * BL], F32, tag="th")   # tanh(C_new) @ 64
            nc.scalar.activation(th[64:80, :], s_st[:, :], AF.Tanh, scale=0.5)

            # 2h = (to+1)*tanh(C_new) -> fwd block t+1, rev block 63-t
            fo_col = BL * (t + 1)
            ro_col = REV + BL * (S - 1 - t)
            h_out = _two_block(comb[EMB:KC, :], fo_col, ro_col, BL)
            nc.vector.scalar_tensor_tensor(
                h_out, tg[64 : 64 + HID, :], 1.0, th[64:80, :], ALU.add, ALU.mult
            )

        # ---------------- projection machinery ----------------
        def ca2_copy(c0, c1, s0c):
            """Fill ca2 cols [c0,c1) from comb blocks starting at s=s0c."""
            n = c1 - c0
            # lefts: fwd h blocks s0c.. (block s holds lefts[s])
            nc.vector.tensor_copy(
                ca2[0:HID, c0:c1], comb[EMB:KC, BL * s0c : BL * s0c + n]
            )
            # rights: rev blocks s0c+1.. (block m holds flipped[m]; block 64=h0)
            nc.sync.dma_start(
                ca2[HID : 2 * HID, c0:c1],
                comb[EMB:KC, REV + BL * (s0c + 1) : REV + BL * (s0c + 1) + n],
            )
            # duplicate both bands at partitions 64:96 (input must start at a
            # 32-aligned partition, so one copy of rows 0:32)
            nc.vector.tensor_copy(ca2[64:96, c0:c1], ca2[0 : 2 * HID, c0:c1])

        cache_tiles = {}

        def a_group(tile_idx, gi):
            band, ls, cw, _, _ = GROUPS[gi]
            r0 = 64 * band
            lhs = ca2[r0 : r0 + 33, tile_idx * 128 : (tile_idx + 1) * 128]
            ps = psP.tile([128, GW], F32, tag="ps", name=f"psA{tile_idx}_{gi}")
            for j0 in range(0, cw, 512):
                jw = min(512, cw - j0)
                nc.tensor.matmul(
                    ps[:, j0 : j0 + jw],
                    lhs,
                    w_sb[r0 : r0 + 33, ls + j0 : ls + j0 + jw],
                    start=True,
                    stop=True,
                )
            eb = cachep.tile([128, GW], BF16, tag="cache", name=f"eb{tile_idx}_{gi}")
            pcol = tile_idx * NG + gi
            nc.scalar.activation(eb[:, :cw], ps[:, :cw], AF.Exp)
            # group sum on DVE (4x bf16 pass with accum) instead of the ACT
            # accumulator: keeps the READ_ACCUMULATOR off the saturated ACT
            sc = scp.tile([128, GW], BF16, tag="sc", name=f"sc{tile_idx}_{gi}")
            nc.vector.tensor_scalar(
                sc[:, :cw], eb[:, :cw], 1.0, 0.0, ALU.mult, ALU.add,
                accum_out=partials[:, pcol : pcol + 1],
            )
            cache_tiles[(tile_idx, gi)] = eb

        def a_norm(tile_idx):
            ssum = obp.tile([128, 1], F32, tag="ssum", name=f"ss{tile_idx}")
            nc.vector.tensor_reduce(
                ssum,
                partials[:, tile_idx * NG : (tile_idx + 1) * NG],
                axis=mybir.AxisListType.X,
                op=ALU.add,
            )
            # bit-hack ln: c_p = float(bits32(S)) * ln2/2^23. The -127*ln2 and
            # mantissa centerings cancel against the cache-side bit-hack.
            nc.vector.tensor_scalar_mul(
                cp[:, tile_idx : tile_idx + 1],
                ssum.bitcast(I32),
                LN2 / (1 << 23),
            )

        def b_group(tile_idx, gi):
            _, _, cw, gs, ow = GROUPS[gi]
            eb = cache_tiles.pop((tile_idx, gi))
            ob = obp.tile([128, GW], BF16, tag="ob", name=f"ob{tile_idx}_{gi}")
            nc.vector.tensor_scalar(
                ob[:, :cw],
                eb[:, :cw].bitcast(U16),
                LN2 / 128.0,
                cp[:, tile_idx : tile_idx + 1],
                ALU.mult,
                ALU.subtract,
            )
            nc.sync.dma_start(
                d_out.ap()[tile_idx * 128 : (tile_idx + 1) * 128, gs : gs + ow],
                ob[:, :ow],
            )

        # ---------------- schedule ----------------
        # scan steps 0..47; tile-0 features ready after step 47
        for t in range(48):
            scan_step(t)
        ca2_copy(0, 128, 16)
        # steps 48..63 with one tile-0 exp group interleaved per 2 steps
        gi_early = 0
        for t in range(48, S):
            scan_step(t)
            if t % 2 == 1:
                a_group(0, gi_early)
                gi_early += 1
        ca2_copy(128, 192, 0)
        ca2_copy(192, 256, 48)

        for gi in range(gi_early, NG):
            a_group(0, gi)
        a_norm(0)
        for gi in range(NG):
            b_group(0, gi)
            a_group(1, gi)
        a_norm(1)
        for gi in range(NG):
            b_group(1, gi)

    _compile_with_ldw_dedup(nc)
    return nc


def _ldw_key(inst):
    a = inst.ins[0]
    return (
        getattr(a, "memref", None),
        getattr(a, "offset", None),
        str(getattr(a, "ap", None)),
        str(getattr(a, "dtype", None)),
        str(inst.perf_mode),
        str(inst.is_transpose),
        str(inst.tile_position),
    )


def _dedup_ldweights(nc):
    """Drop LDWEIGHTS instructions whose weights are already loaded (same AP
    as the previous PE weight load, with no other weight-changing PE
    instruction in between). Same-weight matmuls then issue back-to-back and
    pipeline on the PE instead of serializing on redundant reloads."""
    fn = nc.m.functions[0]
    n_drop = 0
    for bb in fn.blocks:
        out = []
        last_key = None
        carry = []
        for inst in bb.instructions:
            nm = inst.__class__.__name__
            if nm == "InstLdweights":
                si = inst.sync_info
                has_upd = bool(si and si.on_update)
                key = _ldw_key(inst)
                if key == last_key and not has_upd:
                    if si and si.on_wait:
                        carry.extend(si.on_wait)
                    n_drop += 1
                    continue
                last_key = key
            elif nm == "InstMatmult":
                if carry:
                    si = inst.sync_info
                    w = list(si.on_wait) if si and si.on_wait else []
                    si.on_wait = carry + w
                    inst.sync_info = si
                    carry = []
                # self-loading matmuls (f32 / transposes) clobber the array
                if inst.is_transpose or str(
                    getattr(inst.ins[0], "dtype", "")
                ) in ("dt.float32", "dt.float32r"):
                    last_key = None
            out.append(inst)
        assert not carry, "dropped-LDW waits with no following matmul"
        bb.instructions = out
    return n_drop


def _compile_with_ldw_dedup(nc):
    """bacc.Bacc.compile() with an LDWEIGHTS-dedup pass inserted right after
    move_matmul_waits_to_ldweights."""
    nc.insert_bir_kernel_barrier_sem_inc()
    nc.move_matmul_waits_to_ldweights()
    _dedup_ldweights(nc)
    nc.generate_event_semaphores()
    nc.remove_dead_instructions_after_branch()
    nc.validate_blocks()
    nc.dce_regs()
    nc.thread_jumps()
    nc.remove_dead_blocks()
    nc.remove_dead_allocations()
    nc.verify_switch_hints()
    nc.alloc_regs()
    inst_simplify.simplify(nc)
    nc.fuse_regops()
    nc.fuse_blocks()
    nc.replace_nops_with_events()
    for engine in nc.engines:
        nc.fuse_nops(engine)
    nc.remove_dead_nops()
    nc.remove_dangling_data()
    nc.generate_event_semaphores()
    nc.insert_library_loads()
    nc.insert_act_table_loads()
    nc.insert_hostgen_rebases()
    nc.codegen_inst_isa_subclasses()


def host_prep(inputs, ncores=NCORES):
    """Build the per-core input maps from the full problem inputs."""
    import ml_dtypes

    emb = np.ascontiguousarray(np.asarray(inputs["embedding"], dtype=np.float32))
    ib = np.asarray(inputs["input_batch"]).astype(np.int32)          # [S, B]
    W = [np.asarray(inputs[k], dtype=np.float32) for k in ("W_f", "W_i", "W_o", "W_C")]
    b = [np.asarray(inputs[k], dtype=np.float32) for k in ("b_f", "b_i", "b_o", "b_C")]
    W_ho = np.asarray(inputs["W_ho"], dtype=np.float32)
    b_ho = np.asarray(inputs["b_ho"], dtype=np.float32)
    h0 = np.asarray(inputs["initial_hidden"], dtype=np.float32)      # [1, HID]
    c0i = np.asarray(inputs["initial_C"], dtype=np.float32)

    # gate rows: f@0, i@32, o@64, c@96. sigmoid gates become tanh(z/2): W,b
    # halved; all h-columns halved again because the stored hidden is 2h.
    Wc = np.zeros((GP, KC), dtype=np.float32)
    bc = np.zeros((GP, 1), dtype=np.float32)
    for gi, (Wg, bg) in enumerate(zip(W, b)):
        gate_scale = 0.5 if gi < 3 else 1.0
        Wrow = Wg * gate_scale
        Wrow = np.concatenate([Wrow[:, :EMB], Wrow[:, EMB:] * 0.5], axis=1)
        Wc[32 * gi : 32 * gi + HID] = Wrow
        bc[32 * gi : 32 * gi + HID, 0] = bg * gate_scale
    w_combT = np.ascontiguousarray(Wc.T.astype(ml_dtypes.bfloat16))  # [48, 112]

    # projection: rows 0:32 = W_ho.T * 0.5 (features are 2h), row 32 = b_ho
    w_full = np.empty((33, V), dtype=np.float32)
    w_full[0:EMB] = W_ho.T * 0.5
    w_full[EMB] = b_ho
    w_full = w_full.astype(ml_dtypes.bfloat16)
    w_lo = np.ascontiguousarray(w_full[:, :BAND])
    w_hi = np.zeros((33, BAND), dtype=ml_dtypes.bfloat16)
    w_hi[:, : V - BAND] = w_full[:, BAND:]

    h0T = np.ascontiguousarray(
        np.broadcast_to(2.0 * h0.T, (HID, BL))
    ).astype(ml_dtypes.bfloat16)
    s0T = np.ascontiguousarray(
        np.broadcast_to(2.0 * c0i.T, (HID, 2 * BL))
    ).astype(np.float32)

    bl = B // ncores
    in_maps = []
    for c in range(ncores):
        idx = np.ascontiguousarray(
            ib[:, c * bl : (c + 1) * bl].reshape(T, 1)
        )  # token t = s*BL + b
        in_maps.append(
            {
                "emb_table": emb,
                "idx": idx,
                "w_combT": w_combT,
                "b_cell": np.ascontiguousarray(bc),
                "h0": h0T,
                "s0": s0T,
                "w_lo": w_lo,
                "w_hi": w_hi,
            }
        )
    return in_maps


def assemble_output(raw_outs):
    """[T, V] bf16 per core (permuted rows) -> [S, B, V] f32."""
    outs = []
    for r in raw_outs:
        dev = np.asarray(r).reshape(S, BL, V)
        nat = np.empty_like(dev)
        nat[PERM_S] = dev
        outs.append(nat)
    return np.concatenate(outs, axis=1).astype(np.float32)


_NC_CACHE = {}


def kernel(**inputs):
    from concourse.bass_utils import run_bass_kernel_spmd

    if "full" not in _NC_CACHE:
        _NC_CACHE["full"] = build_nc()
    nc = _NC_CACHE["full"]
    in_maps = host_prep(inputs)
    res = run_bass_kernel_spmd(nc, in_maps, core_ids=list(range(NCORES)))
    return assemble_output([r["out"] for r in res.results])


# revision 10
# speedup vs baseline: 1.1117x; 1.1117x over previous
"""BiLSTM language model kernel for Trainium2 (8 NeuronCores), v2.

Sharding: data-parallel over batch (B=32 -> 4 per core). Each core runs the
full bidirectional LSTM scan for its batch slice and the full-vocab output
projection + log-softmax for its tokens locally (no collectives).

v2 design (vs the two-matmul-pass baseline):
  - tanh-only scan: sigmoid(x) = (tanh(x/2)+1)/2 with all scale factors
    folded into host-prepped weights (W halved for f/i/o, h-columns halved
    again since the stored hidden is 2h, cell state tracked as s=2C). The
    whole kernel then uses one ACT table set (exp_and_others: tanh+exp), so
    scan activations and projection exps interleave with no table reloads.
  - bf16 scan weights/state: the [48,112] stationary loads once (LDW dedup),
    per-step matmul is a cheap bf16 2-block stream.
  - single projection pass per token tile: matmul -> ACT Exp (PSUM->SBUF
    bf16 cache + f32 accum = per-group sum). After all groups, pass B is a
    DVE tensor_scalar on the uint16-bitcast cache at 4x: out = bits*(ln2/128)
    - c_p, the bit-hack log. The cache-side and sum-side mantissa biases
    cancel, so c_p = float(bits32(S)) * ln2/2^23 in one DVE op (no ACT Ln).
  - bf16 DRAM output (halved DMA); host casts to f32.
  - token tiles middle-out: tile 0 = s 16..47 is ready at scan step 47, so
    its first exp groups are interleaved into scan steps 48..63; tile 1 =
    s {0..15, 48..63} follows. Phases pipeline as A(0) -> B(0) || A(1) -> B(1).
  - projection weights live in two 33-row bands (partitions 0:33 and 64:97
    of one [97, 26112] bf16 tile) to respect the per-partition SBUF budget;
    the stationary concat features are duplicated at partitions 64:97 so
    base partitions match per band.

DVE 2-input ops need both SBUF inputs at the same base partition; gates land
as tanh_f@0, tanh_i@32, tanh_o@64 with ct parked @32 and tanh(C) @64.
"""

import numpy as np
from contextlib import ExitStack

from concourse import inst_simplify

import concourse.bass as bass
import concourse.mybir as mybir
import concourse.tile as tile
from concourse import bacc
from concourse.masks import make_identity

F32 = mybir.dt.float32
BF16 = mybir.dt.bfloat16
U16 = mybir.dt.uint16
I32 = mybir.dt.int32
AF = mybir.ActivationFunctionType
ALU = mybir.AluOpType

S = 64          # sequence length
B = 32          # full batch
V = 50257       # vocab
HID = 16
EMB = 32
NCORES = 8
BL = B // NCORES          # batch per core = 4
T = S * BL                # tokens per core = 256
KC = EMB + HID            # 48
GP = 112                  # gate rows: tanh_f@0, tanh_i@32, tanh_o@64, tanh_c@96
REV = (S + 1) * BL        # column offset of reverse region in comb = 260
GW = 1536                 # vocab columns per group (3 PSUM banks)
BAND = 26112              # vocab columns in band 0 (= 17 groups)

LN2 = float(np.log(2.0))

# token-tile permutation: dev row r <-> (s = PERM_S[r//4], b = r%4).
# tile 0 (rows 0:128) = s 16..47 -- ready at scan step 47 (middle-out).
PERM_S = list(range(16, 48)) + list(range(0, 16)) + list(range(48, 64))

# groups: (band, local_start, width, global_start, out_width)
GROUPS = []
for g in range(17):
    GROUPS.append((0, g * GW, GW, g * GW, GW))
for g in range(15):
    GROUPS.append((1, g * GW, GW, BAND + g * GW, GW))
# last group padded to even width 1106; the phantom zero-weight column adds
# exp(0)=1 to S (~2e-5 relative) and is not DMA'd out.
GROUPS.append((1, 15 * GW, 1106, BAND + 15 * GW, 1105))
NG = len(GROUPS)          # 33


def _two_block(ap2d, col_a, col_b, width):
    """AP selecting two `width`-column blocks [P, 2, width] of a 2D sbuf AP."""
    base = ap2d
    return bass.AP(
        base.tensor,
        base.offset + col_a,
        [base.ap[0], [col_b - col_a, 2], [1, width]],
    )


def build_nc():
    nc = bacc.Bacc("TRN2", target_bir_lowering=False, debug=False)

    # ---------------- DRAM I/O ----------------
    d_emb = nc.dram_tensor("emb_table", [V, EMB], F32, kind="ExternalInput")
    d_idx = nc.dram_tensor("idx", [T, 1], I32, kind="ExternalInput")
    d_wcomb = nc.dram_tensor("w_combT", [KC, GP], BF16, kind="ExternalInput")
    d_bcell = nc.dram_tensor("b_cell", [GP, 1], F32, kind="ExternalInput")
    d_h0 = nc.dram_tensor("h0", [HID, BL], BF16, kind="ExternalInput")
    d_s0 = nc.dram_tensor("s0", [HID, 2 * BL], F32, kind="ExternalInput")
    d_wlo = nc.dram_tensor("w_lo", [33, BAND], BF16, kind="ExternalInput")
    d_whi = nc.dram_tensor("w_hi", [33, BAND], BF16, kind="ExternalInput")
    d_out = nc.dram_tensor("out", [T, V], BF16, kind="ExternalOutput")

    with tile.TileContext(nc) as tc, ExitStack() as ctx:
        singles = ctx.enter_context(tc.tile_pool(name="singles", bufs=1))
        # projection PSUM first so it gets banks disjoint from the scan pool
        psP = ctx.enter_context(tc.tile_pool(name="psP", bufs=2, space="PSUM"))
        cachep = ctx.enter_context(tc.tile_pool(name="cache", bufs=NG + 1))
        obp = ctx.enter_context(tc.tile_pool(name="ob", bufs=4))
        scp = ctx.enter_context(tc.tile_pool(name="scr", bufs=2))

        # gather inputs first: the idx DMAs lead the sync queue and the
        # indirect gathers lead the gpsimd queue, so the embedding rows are
        # in flight while everything else initializes.
        gat = ctx.enter_context(tc.tile_pool(name="pgather", bufs=2))
        idx_sbs, embgs = [], []
        for half in range(2):
            idx_sb = gat.tile([128, 1], I32, tag="idx", name=f"idx{half}")
            nc.sync.dma_start(idx_sb, d_idx.ap()[half * 128 : (half + 1) * 128, :])
            idx_sbs.append(idx_sb)
        for half in range(2):
            embg = gat.tile([128, EMB], F32, tag="embg", name=f"embg{half}")
            nc.gpsimd.indirect_dma_start(
                out=embg,
                out_offset=None,
                in_=d_emb.ap(),
                in_offset=bass.IndirectOffsetOnAxis(ap=idx_sbs[half][:, :1], axis=0),
            )
            embgs.append(embg)

        ident = singles.tile([128, 128], F32)
        make_identity(nc, ident)

        w_combT = singles.tile([KC, GP], BF16)
        nc.sync.dma_start(w_combT, d_wcomb.ap())
        b_cell = singles.tile([GP, 1], F32)
        nc.sync.dma_start(b_cell, d_bcell.ap())

        # comb: [48, 520] bf16; cols 0..260 fwd blocks 0..64, cols 260..520
        # rev blocks 0..64. rows 0-31 = x (embT), rows 32-47 = 2h state.
        comb = singles.tile([KC, 2 * REV], BF16)
        # concat features, bf16: rows 0-15 lefts, 16-31 rights, 32 ones; the
        # same three bands duplicated at rows 64-96 for band-1 matmuls.
        ca2 = singles.tile([97, T], BF16)
        nc.vector.memset(ca2[32:33, :], 1.0)
        nc.vector.memset(ca2[96:97, :], 1.0)

        # s = 2C state [16, 8] f32 (fwd | rev)
        s_st = singles.tile([HID, 2 * BL], F32)
        nc.sync.dma_start(s_st, d_s0.ap())
        nc.sync.dma_start(comb[EMB:KC, 0:BL], d_h0.ap())            # fwd block 0
        nc.sync.dma_start(comb[EMB:KC, 2 * REV - BL : 2 * REV], d_h0.ap())  # rev 64

        embT = singles.tile([EMB, T], BF16)
        # two-band projection weights: rows 0:33 = vocab [0, 26112),
        # rows 64:97 = vocab [26112, 50257) zero-padded.
        w_sb = singles.tile([97, BAND], BF16)
        nc.scalar.dma_start(w_sb[0:33, :], d_wlo.ap())
        nc.scalar.dma_start(w_sb[64:97, :], d_whi.ap())

        # per-group exp sums; col = tile*NG + g
        partials = singles.tile([128, 2 * NG], F32)
        # c_p = float(bits32(S)) * ln2/2^23 per token, one col per tile
        cp = singles.tile([128, 2], F32)

        # ---------------- embedding transpose via PE ----------------
        with tc.tile_pool(name="ps_misc", bufs=2, space="PSUM") as psm:
            for half in range(2):
                ps_tr = psm.tile([EMB, 128], F32)
                nc.tensor.transpose(ps_tr, embgs[half], ident)
                nc.vector.tensor_copy(embT[:, half * 128 : (half + 1) * 128], ps_tr)

        # x parts of comb: fwd block t = token t; rev block m+1 = token m
        nc.vector.tensor_copy(comb[0:EMB, 0:T], embT)
        nc.vector.tensor_copy(comb[0:EMB, REV + BL : REV + BL + T], embT)

        # ---------------- scan machinery ----------------
        ssb = ctx.enter_context(tc.tile_pool(name="scan_sb", bufs=6))
        sps = ctx.enter_context(tc.tile_pool(name="scan_ps", bufs=2, space="PSUM"))

        def scan_step(t):
            f_col = BL * t                       # fwd block t
            r_col = REV + BL * (S - t)           # rev block 64-t
            rhs = _two_block(comb[:, :], f_col, r_col, BL)
            g_ps = sps.tile([GP, 2 * BL], F32)
            nc.tensor.matmul(g_ps, w_combT, rhs, start=True, stop=True)

            # tg rows: tanh(zf/2)@0, tanh(zi/2)@32, tanh(zo/2)@64
            tg = ssb.tile([96, 2 * BL], F32, tag="tg")
            nc.scalar.activation(
                tg, g_ps[0:96, :], AF.Tanh, bias=b_cell[0:96, :]
            )
            ct = ssb.tile([48, 2 * BL], F32, tag="ct")   # tanh(z_C) @ 32
            nc.scalar.activation(
                ct[32:48, :], g_ps[96:GP, :], AF.Tanh, bias=b_cell[96:GP, :]
            )

            # s_new = 2C_new = 0.5*(tf+1)*s + (ti+1)*ct
            p = ssb.tile([48, 2 * BL], F32, tag="p")
            nc.vector.scalar_tensor_tensor(
                p[32:48, :], tg[0:HID, :], 1.0, s_st[:, :], ALU.add, ALU.mult
            )
            q = ssb.tile([48, 2 * BL], F32, tag="q")
            nc.vector.scalar_tensor_tensor(
                q[32:48, :], tg[32 : 32 + HID, :], 1.0, ct[32:48, :],
                ALU.add, ALU.mult,
            )
            nc.vector.scalar_tensor_tensor(
                s_st[:, :], p[32:48, :], 0.5, q[32:48, :], ALU.mult, ALU.add
            )

            th = ssb.tile([80, 2 * BL], F32, tag="th")   # tanh(C_new) @ 64
            nc.scalar.activation(th[64:80, :], s_st[:, :], AF.Tanh, scale=0.5)

            # 2h = (to+1)*tanh(C_new) -> fwd block t+1, rev block 63-t
            fo_col = BL * (t + 1)
            ro_col = REV + BL * (S - 1 - t)
            h_out = _two_block(comb[EMB:KC, :], fo_col, ro_col, BL)
            nc.vector.scalar_tensor_tensor(
                h_out, tg[64 : 64 + HID, :], 1.0, th[64:80, :], ALU.add, ALU.mult
            )

        # ---------------- projection machinery ----------------
        def ca2_copy(c0, c1, s0c):
            """Fill ca2 cols [c0,c1) from comb blocks starting at s=s0c."""
            n = c1 - c0
            # lefts: fwd h blocks s0c.. (block s holds lefts[s])
            nc.vector.tensor_copy(
                ca2[0:HID, c0:c1], comb[EMB:KC, BL * s0c : BL * s0c + n]
            )
            # rights: rev blocks s0c+1.. (block m holds flipped[m]; block 64=h0)
            nc.sync.dma_start(
                ca2[HID : 2 * HID, c0:c1],
                comb[EMB:KC, REV + BL * (s0c + 1) : REV + BL * (s0c + 1) + n],
            )
            # duplicate both bands at partitions 64:96 (input must start at a
            # 32-aligned partition, so one copy of rows 0:32)
            nc.vector.tensor_copy(ca2[64:96, c0:c1], ca2[0 : 2 * HID, c0:c1])

        cache_tiles = {}

        def a_group(tile_idx, gi):
            band, ls, cw, _, _ = GROUPS[gi]
            r0 = 64 * band
            lhs = ca2[r0 : r0 + 33, tile_idx * 128 : (tile_idx + 1) * 128]
            ps = psP.tile([128, GW], F32, tag="ps", name=f"psA{tile_idx}_{gi}")
            for j0 in range(0, cw, 512):
                jw = min(512, cw - j0)
                nc.tensor.matmul(
                    ps[:, j0 : j0 + jw],
                    lhs,
                    w_sb[r0 : r0 + 33, ls + j0 : ls + j0 + jw],
                    start=True,
                    stop=True,
                )
            eb = cachep.tile([128, GW], BF16, tag="cache", name=f"eb{tile_idx}_{gi}")
            pcol = tile_idx * NG + gi
            nc.scalar.activation(
                eb[:, :cw], ps[:, :cw], AF.Exp,
                accum_out=partials[:, pcol : pcol + 1],
            )
            cache_tiles[(tile_idx, gi)] = eb

        def a_norm(tile_idx):
            ssum = obp.tile([128, 1], F32, tag="ssum", name=f"ss{tile_idx}")
            nc.vector.tensor_reduce(
                ssum,
                partials[:, tile_idx * NG : (tile_idx + 1) * NG],
                axis=mybir.AxisListType.X,
                op=ALU.add,
            )
            # bit-hack ln: c_p = float(bits32(S)) * ln2/2^23. The -127*ln2 and
            # mantissa centerings cancel against the cache-side bit-hack.
            nc.vector.tensor_scalar_mul(
                cp[:, tile_idx : tile_idx + 1],
                ssum.bitcast(I32),
                LN2 / (1 << 23),
            )

        def b_group(tile_idx, gi):
            _, _, cw, gs, ow = GROUPS[gi]
            eb = cache_tiles.pop((tile_idx, gi))
            ob = obp.tile([128, GW], BF16, tag="ob", name=f"ob{tile_idx}_{gi}")
            nc.vector.tensor_scalar(
                ob[:, :cw],
                eb[:, :cw].bitcast(U16),
                LN2 / 128.0,
                cp[:, tile_idx : tile_idx + 1],
                ALU.mult,
                ALU.subtract,
            )
            nc.sync.dma_start(
                d_out.ap()[tile_idx * 128 : (tile_idx + 1) * 128, gs : gs + ow],
                ob[:, :ow],
            )

        # ---------------- schedule ----------------
        # scan steps 0..47; tile-0 features ready after step 47
        for t in range(48):
            scan_step(t)
        ca2_copy(0, 128, 16)
        # steps 48..63 with one tile-0 exp group interleaved per 2 steps
        gi_early = 0
        for t in range(48, S):
            scan_step(t)
            if t % 2 == 1:
                a_group(0, gi_early)
                gi_early += 1
        ca2_copy(128, 192, 0)
        ca2_copy(192, 256, 48)

        for gi in range(gi_early, NG):
            a_group(0, gi)
        a_norm(0)
        for gi in range(NG):
            b_group(0, gi)
            a_group(1, gi)
        a_norm(1)
        for gi in range(NG):
            b_group(1, gi)

    _compile_with_ldw_dedup(nc)
    return nc


def _ldw_key(inst):
    a = inst.ins[0]
    return (
        getattr(a, "memref", None),
        getattr(a, "offset", None),
        str(getattr(a, "ap", None)),
        str(getattr(a, "dtype", None)),
        str(inst.perf_mode),
        str(inst.is_transpose),
        str(inst.tile_position),
    )


def _dedup_ldweights(nc):
    """Drop LDWEIGHTS instructions whose weights are already loaded (same AP
    as the previous PE weight load, with no other weight-changing PE
    instruction in between). Same-weight matmuls then issue back-to-back and
    pipeline on the PE instead of serializing on redundant reloads."""
    fn = nc.m.functions[0]
    n_drop = 0
    for bb in fn.blocks:
        out = []
        last_key = None
        carry = []
        for inst in bb.instructions:
            nm = inst.__class__.__name__
            if nm == "InstLdweights":
                si = inst.sync_info
                has_upd = bool(si and si.on_update)
                key = _ldw_key(inst)
                if key == last_key and not has_upd:
                    if si and si.on_wait:
                        carry.extend(si.on_wait)
                    n_drop += 1
                    continue
                last_key = key
            elif nm == "InstMatmult":
                if carry:
                    si = inst.sync_info
                    w = list(si.on_wait) if si and si.on_wait else []
                    si.on_wait = carry + w
                    inst.sync_info = si
                    carry = []
                # self-loading matmuls (f32 / transposes) clobber the array
                if inst.is_transpose or str(
                    getattr(inst.ins[0], "dtype", "")
                ) in ("dt.float32", "dt.float32r"):
                    last_key = None
            out.append(inst)
        assert not carry, "dropped-LDW waits with no following matmul"
        bb.instructions = out
    return n_drop


def _compile_with_ldw_dedup(nc):
    """bacc.Bacc.compile() with an LDWEIGHTS-dedup pass inserted right after
    move_matmul_waits_to_ldweights."""
    nc.insert_bir_kernel_barrier_sem_inc()
    nc.move_matmul_waits_to_ldweights()
    _dedup_ldweights(nc)
    nc.generate_event_semaphores()
    nc.remove_dead_instructions_after_branch()
    nc.validate_blocks()
    nc.dce_regs()
    nc.thread_jumps()
    nc.remove_dead_blocks()
    nc.remove_dead_allocations()
    nc.verify_switch_hints()
    nc.alloc_regs()
    inst_simplify.simplify(nc)
    nc.fuse_regops()
    nc.fuse_blocks()
    nc.replace_nops_with_events()
    for engine in nc.engines:
        nc.fuse_nops(engine)
    nc.remove_dead_nops()
    nc.remove_dangling_data()
    nc.generate_event_semaphores()
    nc.insert_library_loads()
    nc.insert_act_table_loads()
    nc.insert_hostgen_rebases()
    nc.codegen_inst_isa_subclasses()


def host_prep(inputs, ncores=NCORES):
    """Build the per-core input maps from the full problem inputs."""
    import ml_dtypes

    emb = np.ascontiguousarray(np.asarray(inputs["embedding"], dtype=np.float32))
    ib = np.asarray(inputs["input_batch"]).astype(np.int32)          # [S, B]
    W = [np.asarray(inputs[k], dtype=np.float32) for k in ("W_f", "W_i", "W_o", "W_C")]
    b = [np.asarray(inputs[k], dtype=np.float32) for k in ("b_f", "b_i", "b_o", "b_C")]
    W_ho = np.asarray(inputs["W_ho"], dtype=np.float32)
    b_ho = np.asarray(inputs["b_ho"], dtype=np.float32)
    h0 = np.asarray(inputs["initial_hidden"], dtype=np.float32)      # [1, HID]
    c0i = np.asarray(inputs["initial_C"], dtype=np.float32)

    # gate rows: f@0, i@32, o@64, c@96. sigmoid gates become tanh(z/2): W,b
    # halved; all h-columns halved again because the stored hidden is 2h.
    Wc = np.zeros((GP, KC), dtype=np.float32)
    bc = np.zeros((GP, 1), dtype=np.float32)
    for gi, (Wg, bg) in enumerate(zip(W, b)):
        gate_scale = 0.5 if gi < 3 else 1.0
        Wrow = Wg * gate_scale
        Wrow = np.concatenate([Wrow[:, :EMB], Wrow[:, EMB:] * 0.5], axis=1)
        Wc[32 * gi : 32 * gi + HID] = Wrow
        bc[32 * gi : 32 * gi + HID, 0] = bg * gate_scale
    w_combT = np.ascontiguousarray(Wc.T.astype(ml_dtypes.bfloat16))  # [48, 112]

    # projection: rows 0:32 = W_ho.T * 0.5 (features are 2h), row 32 = b_ho
    w_full = np.empty((33, V), dtype=np.float32)
    w_full[0:EMB] = W_ho.T * 0.5
    w_full[EMB] = b_ho
    w_full = w_full.astype(ml_dtypes.bfloat16)
    w_lo = np.ascontiguousarray(w_full[:, :BAND])
    w_hi = np.zeros((33, BAND), dtype=ml_dtypes.bfloat16)
    w_hi[:, : V - BAND] = w_full[:, BAND:]

    h0T = np.ascontiguousarray(
        np.broadcast_to(2.0 * h0.T, (HID, BL))
    ).astype(ml_dtypes.bfloat16)
    s0T = np.ascontiguousarray(
        np.broadcast_to(2.0 * c0i.T, (HID, 2 * BL))
    ).astype(np.float32)

    bl = B // ncores
    in_maps = []
    for c in range(ncores):
        idx = np.ascontiguousarray(
            ib[:, c * bl : (c + 1) * bl].reshape(T, 1)
        )  # token t = s*BL + b
        in_maps.append(
            {
                "emb_table": emb,
                "idx": idx,
                "w_combT": w_combT,
                "b_cell": np.ascontiguousarray(bc),
                "h0": h0T,
                "s0": s0T,
                "w_lo": w_lo,
                "w_hi": w_hi,
            }
        )
    return in_maps


def assemble_output(raw_outs):
    """[T, V] bf16 per core (permuted rows) -> [S, B, V] f32."""
    outs = []
    for r in raw_outs:
        dev = np.asarray(r).reshape(S, BL, V)
        nat = np.empty_like(dev)
        nat[PERM_S] = dev
        outs.append(nat)
    return np.concatenate(outs, axis=1).astype(np.float32)


_NC_CACHE = {}


def kernel(**inputs):
    from concourse.bass_utils import run_bass_kernel_spmd

    if "full" not in _NC_CACHE:
        _NC_CACHE["full"] = build_nc()
    nc = _NC_CACHE["full"]
    in_maps = host_prep(inputs)
    res = run_bass_kernel_spmd(nc, in_maps, core_ids=list(range(NCORES)))
    return assemble_output([r["out"] for r in res.results])


# revision 13
# speedup vs baseline: 1.1128x; 1.0010x over previous
"""BiLSTM language model kernel for Trainium2 (8 NeuronCores), v2.

Sharding: data-parallel over batch (B=32 -> 4 per core). Each core runs the
full bidirectional LSTM scan for its batch slice and the full-vocab output
projection + log-softmax for its tokens locally (no collectives).

v2 design (vs the two-matmul-pass baseline):
  - tanh-only scan: sigmoid(x) = (tanh(x/2)+1)/2 with all scale factors
    folded into host-prepped weights (W halved for f/i/o, h-columns halved
    again since the stored hidden is 2h, cell state tracked as s=2C). The
    whole kernel then uses one ACT table set (exp_and_others: tanh+exp), so
    scan activations and projection exps interleave with no table reloads.
  - bf16 scan weights/state: the [48,112] stationary loads once (LDW dedup),
    per-step matmul is a cheap bf16 2-block stream.
  - single projection pass per token tile: matmul -> ACT Exp (PSUM->SBUF
    bf16 cache + f32 accum = per-group sum). After all groups, pass B is a
    DVE tensor_scalar on the uint16-bitcast cache at 4x: out = bits*(ln2/128)
    - c_p, the bit-hack log. The cache-side and sum-side mantissa biases
    cancel, so c_p = float(bits32(S)) * ln2/2^23 in one DVE op (no ACT Ln).
  - bf16 DRAM output (halved DMA); host casts to f32.
  - token tiles middle-out: tile 0 = s 16..47 is ready at scan step 47, so
    its first exp groups are interleaved into scan steps 48..63; tile 1 =
    s {0..15, 48..63} follows. Phases pipeline as A(0) -> B(0) || A(1) -> B(1).
  - projection weights live in two 33-row bands (partitions 0:33 and 64:97
    of one [97, 26112] bf16 tile) to respect the per-partition SBUF budget;
    the stationary concat features are duplicated at partitions 64:97 so
    base partitions match per band.

DVE 2-input ops need both SBUF inputs at the same base partition; gates land
as tanh_f@0, tanh_i@32, tanh_o@64 with ct parked @32 and tanh(C) @64.
"""

import numpy as np
from contextlib import ExitStack

from concourse import inst_simplify

import concourse.bass as bass
import concourse.mybir as mybir
import concourse.tile as tile
from concourse import bacc
from concourse.masks import make_identity

F32 = mybir.dt.float32
BF16 = mybir.dt.bfloat16
U16 = mybir.dt.uint16
I32 = mybir.dt.int32
AF = mybir.ActivationFunctionType
ALU = mybir.AluOpType

S = 64          # sequence length
B = 32          # full batch
V = 50257       # vocab
HID = 16
EMB = 32
NCORES = 8
BL = B // NCORES          # batch per core = 4
T = S * BL                # tokens per core = 256
KC = EMB + HID            # 48
GP = 112                  # gate rows: tanh_f@0, tanh_i@32, tanh_o@64, tanh_c@96
REV = (S + 1) * BL        # column offset of reverse region in comb = 260
GW = 1536                 # vocab columns per group (3 PSUM banks)
BAND = 26112              # vocab columns in band 0 (= 17 groups)

LN2 = float(np.log(2.0))

# token-tile permutation: dev row r <-> (s = PERM_S[r//4], b = r%4).
# tile 0 (rows 0:128) = s 16..47 -- ready at scan step 47 (middle-out).
PERM_S = list(range(16, 48)) + list(range(0, 16)) + list(range(48, 64))

# groups: (band, local_start, width, global_start, out_width)
GROUPS = []
for g in range(17):
    GROUPS.append((0, g * GW, GW, g * GW, GW))
for g in range(15):
    GROUPS.append((1, g * GW, GW, BAND + g * GW, GW))
# last group padded to even width 1106; the phantom zero-weight column adds
# exp(0)=1 to S (~2e-5 relative) and is not DMA'd out.
GROUPS.append((1, 15 * GW, 1106, BAND + 15 * GW, 1105))
NG = len(GROUPS)          # 33


def _two_block(ap2d, col_a, col_b, width):
    """AP selecting two `width`-column blocks [P, 2, width] of a 2D sbuf AP."""
    base = ap2d
    return bass.AP(
        base.tensor,
        base.offset + col_a,
        [base.ap[0], [col_b - col_a, 2], [1, width]],
    )


def build_nc():
    nc = bacc.Bacc("TRN2", target_bir_lowering=False, debug=False)

    # ---------------- DRAM I/O ----------------
    d_emb = nc.dram_tensor("emb_table", [V, EMB], F32, kind="ExternalInput")
    d_idx = nc.dram_tensor("idx", [T, 1], I32, kind="ExternalInput")
    d_wcomb = nc.dram_tensor("w_combT", [KC, GP], BF16, kind="ExternalInput")
    d_bcell = nc.dram_tensor("b_cell", [GP, 1], F32, kind="ExternalInput")
    d_h0 = nc.dram_tensor("h0", [HID, BL], BF16, kind="ExternalInput")
    d_s0 = nc.dram_tensor("s0", [HID, 2 * BL], F32, kind="ExternalInput")
    d_wlo = nc.dram_tensor("w_lo", [33, BAND], BF16, kind="ExternalInput")
    d_whi = nc.dram_tensor("w_hi", [33, BAND], BF16, kind="ExternalInput")
    d_out = nc.dram_tensor("out", [T, V], BF16, kind="ExternalOutput")

    with tile.TileContext(nc) as tc, ExitStack() as ctx:
        singles = ctx.enter_context(tc.tile_pool(name="singles", bufs=1))
        # projection PSUM first so it gets banks disjoint from the scan pool
        psP = ctx.enter_context(tc.tile_pool(name="psP", bufs=2, space="PSUM"))
        cachep = ctx.enter_context(tc.tile_pool(name="cache", bufs=NG + 1))
        obp = ctx.enter_context(tc.tile_pool(name="ob", bufs=4))
        scp = ctx.enter_context(tc.tile_pool(name="scr", bufs=2))

        # gather inputs first, in four 64-token segments ordered so the
        # bidirectional scan can start after the first two: (s 0..15,
        # s 48..63) feed scan steps 0..15, then (s 16..31, s 32..47). Each
        # 64-row indirect gather costs ~6us of SWDGE descriptor generation,
        # so the later segments pipeline under the early scan steps.
        SEGS = [0, 3, 1, 2]                      # segment k = tokens 64k..64k+64
        gat = ctx.enter_context(tc.tile_pool(name="pgather", bufs=8))
        idx_sbs, embgs = {}, {}
        for seg in SEGS:
            idx_sb = gat.tile([64, 1], I32, tag=f"idx{seg}", name=f"idx{seg}")
            nc.sync.dma_start(idx_sb, d_idx.ap()[seg * 64 : (seg + 1) * 64, :])
            idx_sbs[seg] = idx_sb
        for seg in SEGS:
            embg = gat.tile([64, EMB], F32, tag=f"embg{seg}", name=f"embg{seg}")
            nc.gpsimd.indirect_dma_start(
                out=embg,
                out_offset=None,
                in_=d_emb.ap(),
                in_offset=bass.IndirectOffsetOnAxis(ap=idx_sbs[seg][:, :1], axis=0),
            )
            embgs[seg] = embg

        ident = singles.tile([128, 128], F32)
        make_identity(nc, ident)

        w_combT = singles.tile([KC, GP], BF16)
        nc.sync.dma_start(w_combT, d_wcomb.ap())
        b_cell = singles.tile([GP, 1], F32)
        nc.sync.dma_start(b_cell, d_bcell.ap())

        # comb: [48, 520] bf16; cols 0..260 fwd blocks 0..64, cols 260..520
        # rev blocks 0..64. rows 0-31 = x (embT), rows 32-47 = 2h state.
        comb = singles.tile([KC, 2 * REV], BF16)
        # concat features, bf16: rows 0-15 lefts, 16-31 rights, 32 ones; the
        # same three bands duplicated at rows 64-96 for band-1 matmuls.
        ca2 = singles.tile([97, T], BF16)
        nc.vector.memset(ca2[32:33, :], 1.0)
        nc.vector.memset(ca2[96:97, :], 1.0)

        # s = 2C state [16, 8] f32 (fwd | rev)
        s_st = singles.tile([HID, 2 * BL], F32)
        nc.sync.dma_start(s_st, d_s0.ap())
        nc.sync.dma_start(comb[EMB:KC, 0:BL], d_h0.ap())            # fwd block 0
        nc.sync.dma_start(comb[EMB:KC, 2 * REV - BL : 2 * REV], d_h0.ap())  # rev 64

        embT = singles.tile([EMB, T], BF16)
        # two-band projection weights: rows 0:33 = vocab [0, 26112),
        # rows 64:97 = vocab [26112, 50257) zero-padded.
        w_sb = singles.tile([97, BAND], BF16)
        nc.scalar.dma_start(w_sb[0:33, :], d_wlo.ap())
        nc.scalar.dma_start(w_sb[64:97, :], d_whi.ap())

        # per-group exp sums; col = tile*NG + g
        partials = singles.tile([128, 2 * NG], F32)
        # c_p = float(bits32(S)) * ln2/2^23 per token, one col per tile
        cp = singles.tile([128, 2], F32)

        # ---------------- embedding transpose via PE, per segment ----------------
        # x parts of comb: fwd block t = token t; rev block m+1 = token m
        with tc.tile_pool(name="ps_misc", bufs=2, space="PSUM") as psm:
            for seg in SEGS:
                ps_tr = psm.tile([EMB, 64], F32)
                nc.tensor.transpose(ps_tr, embgs[seg], ident[0:64, 0:64])
                c0, c1 = seg * 64, (seg + 1) * 64
                nc.vector.tensor_copy(embT[:, c0:c1], ps_tr)
                nc.vector.tensor_copy(comb[0:EMB, c0:c1], embT[:, c0:c1])
                nc.vector.tensor_copy(
                    comb[0:EMB, REV + BL + c0 : REV + BL + c1], embT[:, c0:c1]
                )

        # ---------------- scan machinery ----------------
        ssb = ctx.enter_context(tc.tile_pool(name="scan_sb", bufs=6))
        sps = ctx.enter_context(tc.tile_pool(name="scan_ps", bufs=2, space="PSUM"))

        def scan_step(t):
            f_col = BL * t                       # fwd block t
            r_col = REV + BL * (S - t)           # rev block 64-t
            rhs = _two_block(comb[:, :], f_col, r_col, BL)
            g_ps = sps.tile([GP, 2 * BL], F32)
            nc.tensor.matmul(g_ps, w_combT, rhs, start=True, stop=True)

            # tg rows: tanh(zf/2)@0, tanh(zi/2)@32, tanh(zo/2)@64
            tg = ssb.tile([96, 2 * BL], F32, tag="tg")
            nc.scalar.activation(
                tg, g_ps[0:96, :], AF.Tanh, bias=b_cell[0:96, :]
            )
            ct = ssb.tile([48, 2 * BL], F32, tag="ct")   # tanh(z_C) @ 32
            nc.scalar.activation(
                ct[32:48, :], g_ps[96:GP, :], AF.Tanh, bias=b_cell[96:GP, :]
            )

            # s_new = 2C_new = 0.5*(tf+1)*s + (ti+1)*ct
            p = ssb.tile([48, 2 * BL], F32, tag="p")
            nc.vector.scalar_tensor_tensor(
                p[32:48, :], tg[0:HID, :], 1.0, s_st[:, :], ALU.add, ALU.mult
            )
            q = ssb.tile([48, 2 * BL], F32, tag="q")
            nc.vector.scalar_tensor_tensor(
                q[32:48, :], tg[32 : 32 + HID, :], 1.0, ct[32:48, :],
                ALU.add, ALU.mult,
            )
            nc.vector.scalar_tensor_tensor(
                s_st[:, :], p[32:48, :], 0.5, q[32:48, :], ALU.mult, ALU.add
            )

            th = ssb.tile([80, 2 * BL], F32, tag="th")   # tanh(C_new) @ 64
            nc.scalar.activation(th[64:80, :], s_st[:, :], AF.Tanh, scale=0.5)

            # 2h = (to+1)*tanh(C_new) -> fwd block t+1, rev block 63-t
            fo_col = BL * (t + 1)
            ro_col = REV + BL * (S - 1 - t)
            h_out = _two_block(comb[EMB:KC, :], fo_col, ro_col, BL)
            nc.vector.scalar_tensor_tensor(
                h_out, tg[64 : 64 + HID, :], 1.0, th[64:80, :], ALU.add, ALU.mult
            )

        # ---------------- projection machinery ----------------
        def ca2_copy(c0, c1, s0c):
            """Fill ca2 cols [c0,c1) from comb blocks starting at s=s0c."""
            n = c1 - c0
            # lefts: fwd h blocks s0c.. (block s holds lefts[s])
            nc.vector.tensor_copy(
                ca2[0:HID, c0:c1], comb[EMB:KC, BL * s0c : BL * s0c + n]
            )
            # rights: rev blocks s0c+1.. (block m holds flipped[m]; block 64=h0)
            nc.sync.dma_start(
                ca2[HID : 2 * HID, c0:c1],
                comb[EMB:KC, REV + BL * (s0c + 1) : REV + BL * (s0c + 1) + n],
            )
            # duplicate both bands at partitions 64:96 (input must start at a
            # 32-aligned partition, so one copy of rows 0:32)
            nc.vector.tensor_copy(ca2[64:96, c0:c1], ca2[0 : 2 * HID, c0:c1])

        cache_tiles = {}

        def a_group(tile_idx, gi):
            band, ls, cw, _, _ = GROUPS[gi]
            r0 = 64 * band
            lhs = ca2[r0 : r0 + 33, tile_idx * 128 : (tile_idx + 1) * 128]
            ps = psP.tile([128, GW], F32, tag="ps", name=f"psA{tile_idx}_{gi}")
            for j0 in range(0, cw, 512):
                jw = min(512, cw - j0)
                nc.tensor.matmul(
                    ps[:, j0 : j0 + jw],
                    lhs,
                    w_sb[r0 : r0 + 33, ls + j0 : ls + j0 + jw],
                    start=True,
                    stop=True,
                )
            eb = cachep.tile([128, GW], BF16, tag="cache", name=f"eb{tile_idx}_{gi}")
            pcol = tile_idx * NG + gi
            nc.scalar.activation(
                eb[:, :cw], ps[:, :cw], AF.Exp,
                accum_out=partials[:, pcol : pcol + 1],
            )
            cache_tiles[(tile_idx, gi)] = eb

        def a_norm(tile_idx):
            ssum = obp.tile([128, 1], F32, tag="ssum", name=f"ss{tile_idx}")
            nc.vector.tensor_reduce(
                ssum,
                partials[:, tile_idx * NG : (tile_idx + 1) * NG],
                axis=mybir.AxisListType.X,
                op=ALU.add,
            )
            # bit-hack ln: c_p = float(bits32(S)) * ln2/2^23. The -127*ln2 and
            # mantissa centerings cancel against the cache-side bit-hack.
            nc.vector.tensor_scalar_mul(
                cp[:, tile_idx : tile_idx + 1],
                ssum.bitcast(I32),
                LN2 / (1 << 23),
            )

        def b_group(tile_idx, gi):
            _, _, cw, gs, ow = GROUPS[gi]
            eb = cache_tiles.pop((tile_idx, gi))
            ob = obp.tile([128, GW], BF16, tag="ob", name=f"ob{tile_idx}_{gi}")
            nc.vector.tensor_scalar(
                ob[:, :cw],
                eb[:, :cw].bitcast(U16),
                LN2 / 128.0,
                cp[:, tile_idx : tile_idx + 1],
                ALU.mult,
                ALU.subtract,
            )
            nc.sync.dma_start(
                d_out.ap()[tile_idx * 128 : (tile_idx + 1) * 128, gs : gs + ow],
                ob[:, :ow],
            )

        # ---------------- schedule ----------------
        # scan steps 0..47; tile-0 features ready after step 47
        for t in range(48):
            scan_step(t)
        ca2_copy(0, 128, 16)
        # steps 48..63 with one tile-0 exp group interleaved per 2 steps
        gi_early = 0
        for t in range(48, S):
            scan_step(t)
            if t % 2 == 1:
                a_group(0, gi_early)
                gi_early += 1
        ca2_copy(128, 192, 0)
        ca2_copy(192, 256, 48)

        for gi in range(gi_early, NG):
            a_group(0, gi)
        a_norm(0)
        for gi in range(NG):
            b_group(0, gi)
            a_group(1, gi)
        a_norm(1)
        for gi in range(NG):
            b_group(1, gi)

    _compile_with_ldw_dedup(nc)
    return nc


def _ldw_key(inst):
    a = inst.ins[0]
    return (
        getattr(a, "memref", None),
        getattr(a, "offset", None),
        str(getattr(a, "ap", None)),
        str(getattr(a, "dtype", None)),
        str(inst.perf_mode),
        str(inst.is_transpose),
        str(inst.tile_position),
    )


def _dedup_ldweights(nc):
    """Drop LDWEIGHTS instructions whose weights are already loaded (same AP
    as the previous PE weight load, with no other weight-changing PE
    instruction in between). Same-weight matmuls then issue back-to-back and
    pipeline on the PE instead of serializing on redundant reloads."""
    fn = nc.m.functions[0]
    n_drop = 0
    for bb in fn.blocks:
        out = []
        last_key = None
        carry = []
        for inst in bb.instructions:
            nm = inst.__class__.__name__
            if nm == "InstLdweights":
                si = inst.sync_info
                has_upd = bool(si and si.on_update)
                key = _ldw_key(inst)
                if key == last_key and not has_upd:
                    if si and si.on_wait:
                        carry.extend(si.on_wait)
                    n_drop += 1
                    continue
                last_key = key
            elif nm == "InstMatmult":
                if carry:
                    si = inst.sync_info
                    w = list(si.on_wait) if si and si.on_wait else []
                    si.on_wait = carry + w
                    inst.sync_info = si
                    carry = []
                # self-loading matmuls (f32 / transposes) clobber the array
                if inst.is_transpose or str(
                    getattr(inst.ins[0], "dtype", "")
                ) in ("dt.float32", "dt.float32r"):
                    last_key = None
            out.append(inst)
        assert not carry, "dropped-LDW waits with no following matmul"
        bb.instructions = out
    return n_drop


def _compile_with_ldw_dedup(nc):
    """bacc.Bacc.compile() with an LDWEIGHTS-dedup pass inserted right after
    move_matmul_waits_to_ldweights."""
    nc.insert_bir_kernel_barrier_sem_inc()
    nc.move_matmul_waits_to_ldweights()
    _dedup_ldweights(nc)
    nc.generate_event_semaphores()
    nc.remove_dead_instructions_after_branch()
    nc.validate_blocks()
    nc.dce_regs()
    nc.thread_jumps()
    nc.remove_dead_blocks()
    nc.remove_dead_allocations()
    nc.verify_switch_hints()
    nc.alloc_regs()
    inst_simplify.simplify(nc)
    nc.fuse_regops()
    nc.fuse_blocks()
    nc.replace_nops_with_events()
    for engine in nc.engines:
        nc.fuse_nops(engine)
    nc.remove_dead_nops()
    nc.remove_dangling_data()
    nc.generate_event_semaphores()
    nc.insert_library_loads()
    nc.insert_act_table_loads()
    nc.insert_hostgen_rebases()
    nc.codegen_inst_isa_subclasses()


def host_prep(inputs, ncores=NCORES):
    """Build the per-core input maps from the full problem inputs."""
    import ml_dtypes

    emb = np.ascontiguousarray(np.asarray(inputs["embedding"], dtype=np.float32))
    ib = np.asarray(inputs["input_batch"]).astype(np.int32)          # [S, B]
    W = [np.asarray(inputs[k], dtype=np.float32) for k in ("W_f", "W_i", "W_o", "W_C")]
    b = [np.asarray(inputs[k], dtype=np.float32) for k in ("b_f", "b_i", "b_o", "b_C")]
    W_ho = np.asarray(inputs["W_ho"], dtype=np.float32)
    b_ho = np.asarray(inputs["b_ho"], dtype=np.float32)
    h0 = np.asarray(inputs["initial_hidden"], dtype=np.float32)      # [1, HID]
    c0i = np.asarray(inputs["initial_C"], dtype=np.float32)

    # gate rows: f@0, i@32, o@64, c@96. sigmoid gates become tanh(z/2): W,b
    # halved; all h-columns halved again because the stored hidden is 2h.
    Wc = np.zeros((GP, KC), dtype=np.float32)
    bc = np.zeros((GP, 1), dtype=np.float32)
    for gi, (Wg, bg) in enumerate(zip(W, b)):
        gate_scale = 0.5 if gi < 3 else 1.0
        Wrow = Wg * gate_scale
        Wrow = np.concatenate([Wrow[:, :EMB], Wrow[:, EMB:] * 0.5], axis=1)
        Wc[32 * gi : 32 * gi + HID] = Wrow
        bc[32 * gi : 32 * gi + HID, 0] = bg * gate_scale
    w_combT = np.ascontiguousarray(Wc.T.astype(ml_dtypes.bfloat16))  # [48, 112]

    # projection: rows 0:32 = W_ho.T * 0.5 (features are 2h), row 32 = b_ho
    w_full = np.empty((33, V), dtype=np.float32)
    w_full[0:EMB] = W_ho.T * 0.5
    w_full[EMB] = b_ho
    w_full = w_full.astype(ml_dtypes.bfloat16)
    w_lo = np.ascontiguousarray(w_full[:, :BAND])
    w_hi = np.zeros((33, BAND), dtype=ml_dtypes.bfloat16)
    w_hi[:, : V - BAND] = w_full[:, BAND:]

    h0T = np.ascontiguousarray(
        np.broadcast_to(2.0 * h0.T, (HID, BL))
    ).astype(ml_dtypes.bfloat16)
    s0T = np.ascontiguousarray(
        np.broadcast_to(2.0 * c0i.T, (HID, 2 * BL))
    ).astype(np.float32)

    bl = B // ncores
    in_maps = []
    for c in range(ncores):
        idx = np.ascontiguousarray(
            ib[:, c * bl : (c + 1) * bl].reshape(T, 1)
        )  # token t = s*BL + b
        in_maps.append(
            {
                "emb_table": emb,
                "idx": idx,
                "w_combT": w_combT,
                "b_cell": np.ascontiguousarray(bc),
                "h0": h0T,
                "s0": s0T,
                "w_lo": w_lo,
                "w_hi": w_hi,
            }
        )
    return in_maps


def assemble_output(raw_outs):
    """[T, V] bf16 per core (permuted rows) -> [S, B, V] f32."""
    outs = []
    for r in raw_outs:
        dev = np.asarray(r).reshape(S, BL, V)
        nat = np.empty_like(dev)
        nat[PERM_S] = dev
        outs.append(nat)
    return np.concatenate(outs, axis=1).astype(np.float32)


_NC_CACHE = {}


def kernel(**inputs):
    from concourse.bass_utils import run_bass_kernel_spmd

    if "full" not in _NC_CACHE:
        _NC_CACHE["full"] = build_nc()
    nc = _NC_CACHE["full"]
    in_maps = host_prep(inputs)
    res = run_bass_kernel_spmd(nc, in_maps, core_ids=list(range(NCORES)))
    return assemble_output([r["out"] for r in res.results])
